# revision 1
# baseline (speedup 1.0000x reference)
"""Trainium2 Bass kernel for nn_NeRFGraph (gnn_message_passing).

Strategy (sharding_hint): nodes are sharded across 8 cores aligned to whole
knn batch groups. 20 groups of 2048 nodes -> cores 0-3 take 3 groups,
cores 4-7 take 2 real groups + 1 dummy (SPMD needs uniform shapes; dummy
output is dropped on the host). MLP weights are replicated (data parallel).

Per-core pipeline, per group g (all layouts are [features(partitions), nodes(free)]):
  1. kNN: scores_ij = 2*x_i.x_j - |x_j|^2 accumulated in fp32 PSUM from four
     bf16 matmuls via an exact hi/lo split (x = a+b+c, 2x = u+v+w, sq in 3
     bf16 terms): score = (a+b).u + (a+b).v + c.u + a.w - sq, error ~1e-5
     (= fp32 level; 0 neighbor flips vs the jax reference on the real data).
     Self always wins top-1 (score_self = |x_i|^2 >= score_ij), so top-3
     neighbors = entries 1..3 of the DVE max8/max_index scan. Exact fp32-level
     selection matches the jax reference (0 flips measured on the real data).
  2. MLP (8 layers + skip at 4) in float32r (TF32-like, 1 cyc/row on PE),
     node-tiles interleaved with the kNN row-tiles so PE fills DVE-scan gaps.
  3. EdgeConv x2, factorized: msg_ij = relu(A_i + C_j) with A = F@(W1a-W1b)+b1,
     C = F@W1b per node; gather C_j with gpsimd ap_gather; then W2 matmul,
     mean over K=3 folded into the next layer's weights (host prescale by 1/3).
  4. rgb = sigmoid(S2 @ w_rgb/3 + b_rgb), sigma from the MLP trunk.
"""

import numpy as np
import ml_dtypes

import concourse.bass as bass
import concourse.tile as tile
from concourse import bacc, mybir, library_config
import concourse.bass_utils as bass_utils

F32 = mybir.dt.float32
F32R = mybir.dt.float32r
BF16 = mybir.dt.bfloat16
U16 = mybir.dt.uint16
I16 = mybir.dt.int16
NPBF = ml_dtypes.bfloat16

# problem constants (hardcoded per contract)
B = 40960
NG = 20
DXYZ = 63
DDIR = 27
W = 256
H = 128  # W // 2
KNN = 3

N_CORES = 8
GPC = 3                      # groups per core (SPMD-uniform)
G = B // NG                  # 2048 nodes per group
NODES = GPC * G              # 6144 nodes per core
NT = G // 512                # node tiles of 512 per group
MT = G // 128                # row tiles of 128 per group (knn)
ECCH = 256                   # nodes per edge-conv chunk
ECM = ECCH * KNN             # messages per chunk (768)

_STATE: dict = {}


def _build_nc(reps=1):
    nc = bacc.Bacc(
        "TRN2",
        target_bir_lowering=False,
        debug=False,
        enable_asserts=False,
        num_devices=N_CORES,
    )
    d = {}

    def inp(name, shape, dt=F32):
        d[name] = nc.dram_tensor(name, list(shape), dt, kind="ExternalInput").ap()

    inp("xt", (91, NODES))          # rows 0-62 xyz, 63 ones, 64-90 dir
    inp("tl1", (126, NODES), BF16)  # [a; b]  (xyz hi; xyz lo)
    inp("tl2", (66, NODES), BF16)   # [a; 1; 1; 1]
    inp("tlc", (63, NODES), BF16)   # [c]     (xyz 3rd term)
    inp("tr1", (126, NODES), BF16)  # [u; u]  (2xyz hi, twice)
    inp("tr2", (126, NODES), BF16)  # [v; v]  (2xyz 2nd term, twice)
    inp("tr4", (66, NODES), BF16)   # [w; s1; s2; s3]  (2xyz 3rd term; -sq in 3 bf16 terms)
    inp("w0", (DXYZ, W)); inp("b0", (W, 1))
    inp("wmid", (6, W, W))          # [layer, in, out]
    inp("bmid", (6, W, 1))
    inp("wskip", (DXYZ + W, W)); inp("bskip", (W, 1))
    inp("wfin", (W, W)); inp("bfin", (W, 1))
    inp("wsig", (W, 1)); inp("bsig", (1, 1))
    inp("aw1", (W + DDIR, H)); inp("cw1", (W + DDIR, H)); inp("ab1", (H, 1))
    inp("e1w2", (H, H)); inp("e1b2", (H, 1))
    inp("a2w", (H, H)); inp("c2w", (H, H)); inp("ab2", (H, 1))
    inp("e2w2", (H, H)); inp("e2b2", (H, 1))
    inp("wrgb", (H, 3)); inp("brgb", (3, 1))

    rgb_d = nc.dram_tensor("rgb", [3, NODES], F32, kind="ExternalOutput").ap()
    sig_d = nc.dram_tensor("sig", [1, NODES], F32, kind="ExternalOutput").ap()

    with tile.TileContext(nc) as tc:
        _body(tc, d, rgb_d, sig_d, reps=reps)
    nc.compile()
    return nc


def _body(tc, d, rgb_d, sig_d, reps=1):
    nc = tc.nc
    ctxs = []

    def pool(name, bufs, space="SBUF"):
        p = tc.tile_pool(name=name, bufs=bufs, space=space)
        ctxs.append(p)
        return p.__enter__()

    wstage = pool("wstage", bufs=1)       # f32 staging for weight rounding
    wp = pool("wp", bufs=1)               # persistent rounded weights / biases
    xp = pool("xp", bufs=2)               # per-group inputs (xt)
    xk = pool("xk", bufs=1)               # knn hi/lo inputs (bf16)
    xr = pool("xr", bufs=1)               # rounded per-group inputs
    ap2 = pool("ap2", bufs=2)             # per-group A1/C1 gather tables
    ap1 = pool("ap1", bufs=1)             # per-group S1/A2/C2 tables
    ec = pool("ec", bufs=2)               # edge-conv chunk tiles
    ecg = pool("ecg", bufs=1)             # full-group gather outputs (3x [128,G])
    hp = pool("hp", bufs=2)               # MLP hidden tiles [128,512]
    sp = pool("sp", bufs=4)               # small tiles (vmax/imax)
    op = pool("op", bufs=2)  # outputs               # output staging
    idxp = pool("idxp", bufs=1)
    psp = pool("psp", bufs=1, space="PSUM")    # knn scores [128,2048]
    psm = pool("psm", bufs=2, space="PSUM")    # everything else [128,1024]
    drp = pool("drp", bufs=2, space="DRAM")

    nc.gpsimd.load_library(library_config.ap_gather)

    # ---- per-group input loads (group 0 emitted BEFORE the weight loads so
    # its DMAs are first in the HWDGE queues and kNN can start immediately) ----
    def load_group(g):
        g0 = g * G
        xt = xp.tile([91, G], F32, tag="xt", name="xt")
        nc.sync.dma_start(xt[:], d["xt"][:, g0:g0 + G])
        tl1 = xk.tile([126, G], BF16, tag="tl1", name="tl1")
        nc.sync.dma_start(tl1[:], d["tl1"][:, g0:g0 + G])
        tl2 = xk.tile([66, G], BF16, tag="tl2", name="tl2")
        nc.sync.dma_start(tl2[:], d["tl2"][:, g0:g0 + G])
        tlc = xk.tile([63, G], BF16, tag="tlc", name="tlc")
        nc.sync.dma_start(tlc[:], d["tlc"][:, g0:g0 + G])
        tr1 = xk.tile([126, G], BF16, tag="tr1", name="tr1")
        nc.sync.dma_start(tr1[:], d["tr1"][:, g0:g0 + G])
        tr2 = xk.tile([126, G], BF16, tag="tr2", name="tr2")
        nc.sync.dma_start(tr2[:], d["tr2"][:, g0:g0 + G])
        tr4 = xk.tile([66, G], BF16, tag="tr4", name="tr4")
        nc.sync.dma_start(tr4[:], d["tr4"][:, g0:g0 + G])
        xtr = xr.tile([91, G], F32R, tag="xtr", name="xtr")
        nc.scalar.activation(xtr[:], xt[:], mybir.ActivationFunctionType.Identity)
        return dict(xt=xt, tl1=tl1, tl2=tl2, tlc=tlc, tr1=tr1, tr2=tr2,
                    tr4=tr4, xtr=xtr)

    _pre0 = load_group(0)

    # ---- load + round weights to f32r (one-time) ----
    def load_chunks(src_ap, rows, cols, tag, part_off=0):
        """src_ap: DRAM AP [R, cols]; returns list of rounded chunk tile APs.
        part_off: place the LAST chunk at this base partition (32-aligned)."""
        out = []
        r0 = 0
        for i, r in enumerate(rows):
            last = i == len(rows) - 1
            if last and part_off:
                st = wstage.tile([part_off + r, cols], F32, tag="wstage_p", name="stp")
                nc.sync.dma_start(st[part_off:part_off + r, :], src_ap[r0:r0 + r, :])
                wt = wp.tile([part_off + r, cols], F32R, tag=f"{tag}_{i}", name="wtp")
                nc.scalar.activation(wt[part_off:part_off + r, :],
                                     st[part_off:part_off + r, :],
                                     mybir.ActivationFunctionType.Identity)
                out.append(wt[part_off:part_off + r, :])
            else:
                st = wstage.tile([r, cols], F32, tag="wstage", name="st")
                nc.sync.dma_start(st[:], src_ap[r0:r0 + r, :])
                wt = wp.tile([r, cols], F32R, tag=f"{tag}_{i}", name="wt")
                nc.scalar.activation(wt[:], st[:], mybir.ActivationFunctionType.Identity)
                out.append(wt[:])
            r0 += r
        return out

    def load_b(name, src_ap, rows):
        out = []
        r0 = 0
        for i, r in enumerate(rows):
            bt = wp.tile([r, 1], F32, tag=f"{name}_{i}", name="bt")
            nc.sync.dma_start(bt[:], src_ap[r0:r0 + r, :])
            out.append(bt)
            r0 += r
        return out

    w0 = load_chunks(d["w0"][:], [DXYZ], W, "w0")[0]
    wmid = [load_chunks(d["wmid"][l], [128, 128], W, f"wmid{l}") for l in range(6)]
    wskip = load_chunks(d["wskip"][:], [DXYZ, 128, 128], W, "wskip")
    wfin = load_chunks(d["wfin"][:], [128, 128], W, "wfin")
    wsig = load_chunks(d["wsig"][:], [128, 128], 1, "wsig")
    # dir chunk placed at base partition 64 to match xtr[64:91] (32-aligned)
    aw1 = load_chunks(d["aw1"][:], [128, 128, DDIR], H, "aw1", part_off=64)
    cw1 = load_chunks(d["cw1"][:], [128, 128, DDIR], H, "cw1", part_off=64)
    e1w2 = load_chunks(d["e1w2"][:], [H], H, "e1w2")[0]
    a2w = load_chunks(d["a2w"][:], [H], H, "a2w")[0]
    c2w = load_chunks(d["c2w"][:], [H], H, "c2w")[0]
    e2w2 = load_chunks(d["e2w2"][:], [H], H, "e2w2")[0]
    wrgb = load_chunks(d["wrgb"][:], [H], 3, "wrgb")[0]

    b0 = load_b("b0", d["b0"][:], [128, 128])
    bmid = [load_b(f"bmid{l}", d["bmid"][l], [128, 128]) for l in range(6)]
    bskip = load_b("bskip", d["bskip"][:], [128, 128])
    bfin = load_b("bfin", d["bfin"][:], [128, 128])
    bsig = load_b("bsig", d["bsig"][:], [1])[0]
    ab1 = load_b("ab1", d["ab1"][:], [H])[0]
    e1b2 = load_b("e1b2", d["e1b2"][:], [H])[0]
    ab2 = load_b("ab2", d["ab2"][:], [H])[0]
    e2b2 = load_b("e2b2", d["e2b2"][:], [H])[0]
    brgb = load_b("brgb", d["brgb"][:], [3])[0]

    ACT = mybir.ActivationFunctionType

    for gi in range(GPC * reps):
        g = gi % GPC
        g0 = g * G
        t_in = _pre0 if gi == 0 else load_group(g)
        xt = t_in["xt"]; tl1 = t_in["tl1"]; tl2 = t_in["tl2"]
        tlc = t_in["tlc"]; tr1 = t_in["tr1"]; tr2 = t_in["tr2"]
        tr4 = t_in["tr4"]; xtr = t_in["xtr"]

        nbr = drp.tile([KNN * G], U16, tag="nbr")   # wrapped k-major image
        # addr = k*2048 + r*128 + q  <->  element m of gather list k at [r=m%16, f=m//16]
        nbr3 = nbr[:].rearrange("(k r q) -> q r k", k=KNN, r=16, q=128)
        a1 = ap2.tile([H, G], F32, tag="a1")
        c1 = ap2.tile([H, G], F32, tag="c1")

        def knn_mt(mt):
            ps = psp.tile([128, 2048], F32, tag="ps", name="ps")
            msl = slice(mt * 128, (mt + 1) * 128)
            for nt in range(4):
                osl = slice(nt * 512, (nt + 1) * 512)
                nc.tensor.matmul(ps[:, osl], tl1[:, msl], tr1[:, osl],
                                 start=True, stop=False)
                nc.tensor.matmul(ps[:, osl], tl1[:, msl], tr2[:, osl],
                                 start=False, stop=False)
                nc.tensor.matmul(ps[:, osl], tlc[:, msl], tr1[0:DXYZ, osl],
                                 start=False, stop=False)
                nc.tensor.matmul(ps[:, osl], tl2[:, msl], tr4[:, osl],
                                 start=False, stop=True)
            vmax = sp.tile([128, 8], F32, tag="vmax")
            nc.vector.max(vmax[:], ps[:])
            imax = sp.tile([128, 8], U16, tag="imax")
            nc.vector.max_index(imax[:], vmax[:], ps[:])
            for k in range(KNN):
                nc.sync.dma_start(nbr3[mt * 8:(mt + 1) * 8, :, k], imax[:, 1 + k])

        def mlp_nt(nt):
            n0 = nt * 512
            sl = slice(n0, n0 + 512)

            # L0: [63]->256
            ps = psm.tile([128, 1024], F32, tag="pm", name="ps0")
            nc.tensor.matmul(ps[:, 0:512], w0[:, 0:128], xtr[0:DXYZ, sl], start=True, stop=True)
            nc.tensor.matmul(ps[:, 512:1024], w0[:, 128:256], xtr[0:DXYZ, sl], start=True, stop=True)
            h = [hp.tile([128, 512], F32R, tag=f"h{mh}", name=f"h{mh}") for mh in range(2)]
            nc.scalar.activation(h[0][:], ps[:, 0:512], ACT.Relu, bias=b0[0][:])
            nc.scalar.activation(h[1][:], ps[:, 512:1024], ACT.Relu, bias=b0[1][:])

            # layers 1..7
            m = 0
            for layer in range(1, 8):
                ps = psm.tile([128, 1024], F32, tag="pm", name="psl")
                if layer == 4:
                    bk = bskip
                    for mh in range(2):
                        osl = slice(mh * 512, mh * 512 + 512)
                        mslw = slice(mh * 128, mh * 128 + 128)
                        nc.tensor.matmul(ps[:, osl], wskip[0][:, mslw],
                                         xtr[0:DXYZ, sl], start=True, stop=False)
                        nc.tensor.matmul(ps[:, osl], wskip[1][:, mslw],
                                         h[0][:], start=False, stop=False)
                        nc.tensor.matmul(ps[:, osl], wskip[2][:, mslw],
                                         h[1][:], start=False, stop=True)
                else:
                    wk, bk = wmid[m], bmid[m]
                    m += 1
                    for mh in range(2):
                        osl = slice(mh * 512, mh * 512 + 512)
                        mslw = slice(mh * 128, mh * 128 + 128)
                        nc.tensor.matmul(ps[:, osl], wk[0][:, mslw],
                                         h[0][:], start=True, stop=False)
                        nc.tensor.matmul(ps[:, osl], wk[1][:, mslw],
                                         h[1][:], start=False, stop=True)
                hn = [hp.tile([128, 512], F32R, tag=f"h{mh}", name=f"hn{mh}") for mh in range(2)]
                nc.scalar.activation(hn[0][:], ps[:, 0:512], ACT.Relu, bias=bk[0][:])
                nc.scalar.activation(hn[1][:], ps[:, 512:1024], ACT.Relu, bias=bk[1][:])
                h = hn

            # final (no relu) + sigma
            ps = psm.tile([128, 1024], F32, tag="pm", name="psf")
            pss = psm.tile([1, 512], F32, tag="pm", name="pss")
            for mh in range(2):
                osl = slice(mh * 512, mh * 512 + 512)
                mslw = slice(mh * 128, mh * 128 + 128)
                nc.tensor.matmul(ps[:, osl], wfin[0][:, mslw],
                                 h[0][:], start=True, stop=False)
                nc.tensor.matmul(ps[:, osl], wfin[1][:, mslw],
                                 h[1][:], start=False, stop=True)
            nc.tensor.matmul(pss[0:1, 0:512], wsig[0][:], h[0][:], start=True, stop=False)
            nc.tensor.matmul(pss[0:1, 0:512], wsig[1][:], h[1][:], start=False, stop=True)
            feat = [hp.tile([128, 512], F32R, tag=f"feat{mh}", name=f"feat{mh}") for mh in range(2)]
            nc.scalar.activation(feat[0][:], ps[:, 0:512], ACT.Identity, bias=bfin[0][:])
            nc.scalar.activation(feat[1][:], ps[:, 512:1024], ACT.Identity, bias=bfin[1][:])
            sgt = op.tile([1, 512], F32, tag="sgt")
            nc.scalar.activation(sgt[:], pss[0:1, 0:512], ACT.Identity, bias=bsig[:])
            nc.sync.dma_start(sig_d[:, g0 + n0:g0 + n0 + 512], sgt[:])

            # A1 / C1 over feat(256) + dir(27)
            ps = psm.tile([128, 1024], F32, tag="pm", name="psac")
            for dst_sl, wt in ((slice(0, 512), aw1), (slice(512, 1024), cw1)):
                nc.tensor.matmul(ps[:, dst_sl], wt[0], feat[0][:], start=True, stop=False)
                nc.tensor.matmul(ps[:, dst_sl], wt[1], feat[1][:], start=False, stop=False)
                nc.tensor.matmul(ps[:, dst_sl], wt[2], xtr[64:91, sl], start=False, stop=True)
            nc.scalar.activation(a1[:, sl], ps[:, 0:512], ACT.Identity, bias=ab1[:])
            nc.scalar.activation(c1[:, sl], ps[:, 512:1024], ACT.Copy)

        # interleave knn row-tiles with MLP node-tiles
        for mt in range(MT):
            knn_mt(mt)
            if mt % 4 == 3:
                mlp_nt(mt // 4)

        # wrapped gather indices, k-major: idxw[:, k*128+f] block for neighbor k.
        # One strided DRAM read into partitions 0:16, then replicate to all
        # 16-partition blocks (one per Q7 core) with SBUF->SBUF copies.
        idxw = idxp.tile([128, G * KNN // 16], I16, tag="idxw")
        nbr_r = nbr[:].rearrange("(k r f) -> r k f", k=KNN, r=16, f=128)
        nc.sync.dma_start(
            idxw[0:16, :].rearrange("r (k f) -> r k f", k=KNN),
            nbr_r.bitcast(I16))
        for r in range(1, 8):
            nc.sync.dma_start(idxw[16 * r:16 * r + 16, :], idxw[0:16, :])

        # ---- EdgeConv 1 ----
        s1 = ap1.tile([H, G], F32R, tag="s1")
        g1 = ecg.tile([128, KNN * G], F32, tag="g1")
        for k in range(KNN):
            nc.gpsimd.ap_gather(g1[:, k * G:(k + 1) * G], c1[:],
                                idxw[:, k * 128:(k + 1) * 128], channels=128,
                                num_elems=G, d=1, num_idxs=G)
        for c in range(G // ECCH):
            nsl = slice(c * ECCH, (c + 1) * ECCH)
            msg = ec.tile([128, ECM], F32, tag="msg")
            for k in range(KNN):
                ksl = slice(k * ECCH, (k + 1) * ECCH)
                nc.vector.tensor_add(msg[:, ksl], g1[:, k * G + c * ECCH:k * G + (c + 1) * ECCH],
                                     a1[:, nsl])
            msgr = ec.tile([128, ECM], F32R, tag="msgr")
            nc.scalar.activation(msgr[:], msg[:], ACT.Relu)
            ps = psm.tile([128, 1024], F32, tag="pm", name="psw")
            nc.tensor.matmul(ps[:, 0:512], e1w2[:], msgr[:, 0:512], start=True, stop=True)
            nc.tensor.matmul(ps[:, 512:512 + ECM - 512], e1w2[:], msgr[:, 512:ECM], start=True, stop=True)
            h2 = ec.tile([128, ECM], F32, tag="h2")
            nc.scalar.activation(h2[:], ps[:, 0:ECM], ACT.Relu, bias=e1b2[:])
            tmp = ec.tile([128, ECCH], F32, tag="trio")
            nc.vector.tensor_add(tmp[:], h2[:, 0:ECCH], h2[:, ECCH:2 * ECCH])
            nc.vector.tensor_add(s1[:, nsl], tmp[:], h2[:, 2 * ECCH:3 * ECCH])

        # ---- EdgeConv 2 ----
        a2 = ap1.tile([H, G], F32, tag="a2")
        c2 = ap1.tile([H, G], F32, tag="c2")
        for c in range(NT):
            nsl = slice(c * 512, (c + 1) * 512)
            ps = psm.tile([128, 1024], F32, tag="pm", name="psa2")
            nc.tensor.matmul(ps[:, 0:512], a2w[:], s1[:, nsl], start=True, stop=True)
            nc.tensor.matmul(ps[:, 512:1024], c2w[:], s1[:, nsl], start=True, stop=True)
            nc.scalar.activation(a2[:, nsl], ps[:, 0:512], ACT.Identity, bias=ab2[:])
            nc.scalar.activation(c2[:, nsl], ps[:, 512:1024], ACT.Copy)

        g2 = ecg.tile([128, KNN * G], F32, tag="g1")
        for k in range(KNN):
            nc.gpsimd.ap_gather(g2[:, k * G:(k + 1) * G], c2[:],
                                idxw[:, k * 128:(k + 1) * 128], channels=128,
                                num_elems=G, d=1, num_idxs=G)
        for c in range(G // ECCH):
            nsl = slice(c * ECCH, (c + 1) * ECCH)
            msg = ec.tile([128, ECM], F32, tag="msg")
            for k in range(KNN):
                ksl = slice(k * ECCH, (k + 1) * ECCH)
                nc.vector.tensor_add(msg[:, ksl], g2[:, k * G + c * ECCH:k * G + (c + 1) * ECCH],
                                     a2[:, nsl])
            msgr = ec.tile([128, ECM], F32R, tag="msgr")
            nc.scalar.activation(msgr[:], msg[:], ACT.Relu)
            ps = psm.tile([128, 1024], F32, tag="pm", name="psw2")
            nc.tensor.matmul(ps[:, 0:512], e2w2[:], msgr[:, 0:512], start=True, stop=True)
            nc.tensor.matmul(ps[:, 512:512 + ECM - 512], e2w2[:], msgr[:, 512:ECM], start=True, stop=True)
            h2 = ec.tile([128, ECM], F32, tag="h2")
            nc.scalar.activation(h2[:], ps[:, 0:ECM], ACT.Relu, bias=e2b2[:])
            tmp = ec.tile([128, ECCH], F32, tag="trio")
            nc.vector.tensor_add(tmp[:], h2[:, 0:ECCH], h2[:, ECCH:2 * ECCH])
            s2 = ec.tile([128, ECCH], F32R, tag="s2")
            nc.vector.tensor_add(s2[:], tmp[:], h2[:, 2 * ECCH:3 * ECCH])
            # rgb
            psr = psm.tile([3, ECCH], F32, tag="pm", name="psr")
            nc.tensor.matmul(psr[0:3, 0:ECCH], wrgb[:], s2[:], start=True, stop=True)
            rgt = op.tile([3, ECCH], F32, tag="rgt")
            nc.scalar.activation(rgt[:], psr[0:3, 0:ECCH], ACT.Sigmoid, bias=brgb[:])
            nc.sync.dma_start(rgb_d[:, g0 + c * ECCH:g0 + (c + 1) * ECCH], rgt[:])

    for p in reversed(ctxs):
        p.__exit__(None, None, None)


def _core_groups():
    cg = []
    for c in range(N_CORES):
        if c < 4:
            gs = [3 * c, 3 * c + 1, 3 * c + 2]
        else:
            g0 = 12 + 2 * (c - 4)
            gs = [g0, g0 + 1, g0]  # 3rd slot = dummy repeat
        cg.append(gs)
    return cg


def _prep(inputs):
    x = np.asarray(inputs["x"], dtype=np.float32)
    batch_ids = np.asarray(inputs["batch_ids"])
    perm = np.argsort(batch_ids, kind="stable")
    xs = np.ascontiguousarray(x[perm])

    xyz = xs[:, :DXYZ]
    sq = (xyz * xyz).sum(1, dtype=np.float32)

    w = {k: np.asarray(inputs[k], dtype=np.float32) for k in inputs if k not in ("x", "batch_ids")}
    e1 = w["e1_w1"]
    aw1 = np.ascontiguousarray(e1[:W + DDIR] - e1[W + DDIR:])
    cw1 = np.ascontiguousarray(e1[W + DDIR:])
    e2 = w["e2_w1"]
    a2w = np.ascontiguousarray((e2[:H] - e2[H:]) / 3.0)
    c2w = np.ascontiguousarray(e2[H:] / 3.0)

    shared = {
        "w0": w["w0"], "b0": w["b0"].reshape(W, 1),
        "wmid": w["w_mid"], "bmid": w["b_mid"].reshape(6, W, 1),
        "wskip": w["w_skip"], "bskip": w["b_skip"].reshape(W, 1),
        "wfin": w["w_final"], "bfin": w["b_final"].reshape(W, 1),
        "wsig": w["w_sigma"], "bsig": w["b_sigma"].reshape(1, 1),
        "aw1": aw1, "cw1": cw1, "ab1": w["e1_b1"].reshape(H, 1),
        "e1w2": w["e1_w2"], "e1b2": w["e1_b2"].reshape(H, 1),
        "a2w": a2w, "c2w": c2w, "ab2": w["e2_b1"].reshape(H, 1),
        "e2w2": w["e2_w2"], "e2b2": w["e2_b2"].reshape(H, 1),
        "wrgb": np.ascontiguousarray(w["w_rgb"] / 3.0), "brgb": w["b_rgb"].reshape(3, 1),
    }
    shared = {k: np.ascontiguousarray(v, dtype=np.float32) for k, v in shared.items()}

    in_maps = []
    for gs in _core_groups():
        rows = np.concatenate([np.arange(g * G, (g + 1) * G) for g in gs])
        xc = xs[rows]
        xyzT = np.ascontiguousarray(xc[:, :DXYZ].T)   # [63, NODES] f32
        xt = np.empty((91, NODES), np.float32)
        xt[0:DXYZ] = xyzT
        xt[DXYZ] = 1.0
        xt[DXYZ + 1:] = xc[:, DXYZ:].T

        a = xyzT.astype(NPBF)
        bb = (xyzT - a.astype(np.float32)).astype(NPBF)
        cc = (xyzT - a.astype(np.float32) - bb.astype(np.float32)).astype(NPBF)
        two = 2.0 * xyzT
        u = two.astype(NPBF)
        v = (two - u.astype(np.float32)).astype(NPBF)
        ww = (two - u.astype(np.float32) - v.astype(np.float32)).astype(NPBF)
        nsq = -sq[rows]
        s1 = nsq.astype(NPBF)
        s2 = (nsq - s1.astype(np.float32)).astype(NPBF)
        s3 = (nsq - s1.astype(np.float32) - s2.astype(np.float32)).astype(NPBF)
        ones3 = np.ones((3, NODES), NPBF)

        m = dict(shared)
        m["xt"] = np.ascontiguousarray(xt)
        m["tl1"] = np.ascontiguousarray(np.concatenate([a, bb], 0))
        m["tl2"] = np.ascontiguousarray(np.concatenate([a, ones3], 0))
        m["tlc"] = np.ascontiguousarray(cc)
        m["tr1"] = np.ascontiguousarray(np.concatenate([u, u], 0))
        m["tr2"] = np.ascontiguousarray(np.concatenate([v, v], 0))
        m["tr4"] = np.ascontiguousarray(np.concatenate([ww, s1[None], s2[None], s3[None]], 0))
        in_maps.append(m)
    return in_maps, perm


def _assemble(results, perm):
    out_sorted = np.empty((B, 4), np.float32)
    for c, gs in enumerate(_core_groups()):
        r = results[c]
        for slot, g in enumerate(gs):
            if c >= 4 and slot == 2:
                continue  # dummy
            sl = slice(slot * G, (slot + 1) * G)
            out_sorted[g * G:(g + 1) * G, 0:3] = r["rgb"][:, sl].T
            out_sorted[g * G:(g + 1) * G, 3] = r["sig"][0, sl]
    out = np.empty((B, 4), np.float32)
    out[perm] = out_sorted
    return out


def get_nc(reps=1):
    key = f"nc{reps}"
    if key not in _STATE:
        _STATE[key] = _build_nc(reps)
    return _STATE[key]


def kernel(**inputs) -> np.ndarray:
    nc = get_nc()
    in_maps, perm = _prep(inputs)
    res = bass_utils.run_bass_kernel_spmd(nc, in_maps, core_ids=list(range(N_CORES)))
    return _assemble(res.results, perm)

